# revision 45
# baseline (speedup 1.0000x reference)
"""CenterVLAD Trainium2 kernel.

Reference computation (per batch b, with N = H*W = 1024 pixels, D=32, K=116):
    s = x @ W                    # (N, K)
    a = softmax(s, axis=-1)
    v = x.T @ a + (sum_n a) * C  # (D, K)
    v /= sqrt(sum_d v^2 + eps)   # intra-norm over D
    y = v.flatten()
    y /= sqrt(sum y^2 + eps)     # global L2

Sharding: data-parallel over batch B=32 across 8 cores (4 batches/core),
W and C replicated, no collectives.

Per-core layout: pixels-on-partitions.  Pixel n = 8*p + t lives on partition
p, sub-chunk t in 0..7; chunk (g, q) (t = 4g+q) is the 128-pixel set
{8p + 4g + q}.  The PE transposes 4 chunks at a time ((128,[4,32]) ->
(128,128), all at partition base 0 -- this device path rejects matmuls whose
operands sit at a nonzero partition base, so K=32 row-packing via
tile_position is unusable).  s = x @ W then runs as ONE matmul per transpose
group against a block-diagonal zero-masked W (128 x 4*116): the zero rows
mask out the other three chunks, so a full K=128 contraction yields all four
chunks' s columns side by side.  Softmax runs in pixel-partition layout; the
pooling matmul e.T @ [x*r | r] contracts over pixels giving (116, 33) =
[v1.T | asum] so the intra-normalization runs along the free dim; rsqrt is
exp(-0.5*ln(.)) (one ACT table set; ACT Rsqrt/Sqrt are banned/inaccurate).

Toolchain quirks this kernel works around:
  * walrus here accepts at most ONE sync wait per instruction: dead-end 1x1
    "absorber" matmuls/copies make an engine observe foreign semaphores
    first (pinned before their consumers via add_dep_helper, since the
    scheduler would otherwise sink dead-end ops), SBUF pools are sized so
    per-batch tiles are never reused (no WAR deps), and the Tile kernel-tail
    drain is split into a chain of one-wait drains.
  * EVENT_SEMAPHORE_RANGE_CLEAR and tensor_tensor_reduce fail codegen; the
    sem clear is skipped (fresh NEFF per load) and square+reduce is two ops.
"""

import numpy as np
from contextlib import ExitStack

import concourse.bass as bass
import concourse.tile as tile
from concourse import mybir, masks
from concourse.tile import add_dep_helper

F32 = mybir.dt.float32
AF = mybir.ActivationFunctionType
ALU = mybir.AluOpType

B = 32          # total batches
N = 1024        # H*W pixels per batch
D = 32          # channels
K = 116         # clusters
NCORES = 8
BPC = B // NCORES   # batches per core
T = 8               # pixel sub-chunks per batch (each chunk = 128 pixels)
EPS = 1e-12


def _ap(t, offs_el, dims):
    """Manual AP over tile/dram handle `t`: dims = [[step, count], ...] in
    elements, first dim = partition."""
    base = t[:] if not isinstance(t, bass.AP) else t
    return bass.AP(tensor=base.tensor, offset=base.offset + offs_el, ap=dims)


_JUNK = []


def _absorb(nc, ap):
    """1x1 dummy matmul on the PE whose only role is to make the PE observe
    `ap`'s producer semaphore (walrus allows one sync wait per instruction).
    Writes a dedicated write-only PSUM bank; matmul-after-matmul ordering is
    program order so no extra semaphore appears."""
    return nc.tensor.matmul(_JUNK[0][0:1, 0:1], ap, ap, start=True, stop=True,
                            tile_position=(0, 0))


def _emit(ctx, tc, y_out, x_in, w_in, c_in):
    nc = tc.nc

    singles = ctx.enter_context(tc.tile_pool(name="singles", bufs=1))
    xt_sbp = ctx.enter_context(tc.tile_pool(name="xt_sbp", bufs=8))
    epool = ctx.enter_context(tc.tile_pool(name="epool", bufs=4))
    small = ctx.enter_context(tc.tile_pool(name="small", bufs=8))
    xppool = ctx.enter_context(tc.tile_pool(name="xppool", bufs=4))
    ytp = ctx.enter_context(tc.tile_pool(name="ytp", bufs=4))

    ps_xt = ctx.enter_context(tc.tile_pool(name="ps_xt", bufs=2, space="PSUM"))
    ps_s = ctx.enter_context(tc.tile_pool(name="ps_s", bufs=1, space="PSUM"))
    ps_v = ctx.enter_context(tc.tile_pool(name="ps_v", bufs=1, space="PSUM"))
    ps_b = ctx.enter_context(tc.tile_pool(name="ps_b", bufs=2, space="PSUM"))
    ps_j = ctx.enter_context(tc.tile_pool(name="ps_j", bufs=1, space="PSUM"))
    _JUNK.clear()
    junk_ps = ps_j.tile([1, 1], F32, tag="junk")
    _JUNK.append(junk_ps)
    djunk = singles.tile([1, 64], F32)
    dj_i = [0]

    def dve_absorb(ap):
        # absorber for the DVE; each writes its own djunk element so
        # consecutive absorbers don't WAW-chain each other
        i = dj_i[0] % 64
        dj_i[0] += 1
        return nc.vector.tensor_copy(djunk[0:1, i:i + 1], ap)

    def order(consumer, *absorbers):
        # absorbers are dead-end ops; the priority scheduler would otherwise
        # sink them below the very instructions they must precede
        for a in absorbers:
            add_dep_helper(consumer.ins, a.ins, reason="absorber ordering")

    # ---- constants -------------------------------------------------------
    identity = singles.tile([128, 128], F32)
    masks.make_identity(nc, identity[:])

    ones_t = singles.tile([128, K], F32)
    nc.vector.memset(ones_t[:], 1.0)

    epsb = singles.tile([128, 1], F32)
    nc.vector.memset(epsb[:], EPS)

    w_sb = singles.tile([D, K], F32)
    nc.sync.dma_start(out=w_sb[:], in_=w_in[:, :])
    c_sb = singles.tile([D, K], F32)
    nc.sync.dma_start(out=c_sb[:], in_=c_in[:, :])

    # PE pre-observes the constant producers so later matmuls carry at most
    # one sync wait each.
    a_id = _absorb(nc, identity[0:1, 0:1])
    a_w = _absorb(nc, w_sb[0:1, 0:1])

    # replicate W onto the 4 partition groups (column packing works on this
    # device path), then scatter into the block-diagonal zero-masked W
    w4_ps = ps_b.tile([128, K], F32, tag="phb")
    for q in range(4):
        mm_w4 = nc.tensor.matmul(
            w4_ps[32 * q:32 * q + 32, :], identity[0:D, 0:D], w_sb[:],
            start=True, stop=True, tile_position=(0, 32 * q))
        order(mm_w4, a_id, a_w)
    wblk = singles.tile([128, 4 * K], F32)
    nc.vector.memset(wblk[:], 0.0)
    ajunk = singles.tile([1, 8], F32)
    ab1 = nc.scalar.copy(ajunk[0:1, 0:1], wblk[0:1, 0:1])        # DVE memset
    ab2 = nc.scalar.copy(ajunk[0:1, 1:2], w4_ps[96:97, 0:1])     # PE matmuls
    for q in range(4):
        cp = nc.scalar.copy(wblk[32 * q:32 * q + 32, K * q:K * q + K],
                            w4_ps[32 * q:32 * q + 32, :])
        order(cp, ab1, ab2)

    ct_ps = ps_b.tile([K, D], F32, tag="phb")
    mm_ct = nc.tensor.transpose(ct_ps[:], c_sb[:], identity[0:D, 0:D])
    order(mm_ct, a_id)
    ct_sb = singles.tile([K, D], F32)
    nc.vector.tensor_copy(ct_sb[:], ct_ps[:])

    # ---- input load: one 512 KiB DMA for all 4 batches (fewer DMA lanes
    # keeps the kernel-tail drain's wait list small) -----------------------
    xall = singles.tile([128, BPC, T, D], F32)
    nc.sync.dma_start(
        out=xall[:],
        in_=x_in.rearrange("b (p t) d -> p b t d", t=T))
    xx = [xall[:, b] for b in range(BPC)]

    # ---- per-core staging for the normalization phase --------------------
    vstage = singles.tile([K, BPC, D], F32)    # [v1.T + asum*C.T] per batch
    ss = singles.tile([K, BPC], F32)           # intra-norm sum of squares
    ysb = singles.tile([D, BPC, K], F32)       # final output staging

    # ---- phase A: per batch ---------------------------------------------
    prev_xt = None
    for b in range(BPC):
        xb = xx[b]

        # transpose 4 chunks at a time: (128 pix, [t4,d32]) -> (128, 128 pix)
        pre_abs = []
        if b == 0:
            pre_abs.append(_absorb(nc, xb[0:1, 0, 0:1]))   # input-DMA lane
        if prev_xt is not None:
            # absorb the psum-slot WAR on last batch's transpose readers
            pre_abs.append(_absorb(nc, prev_xt[0][0:1, 0:1]))
            pre_abs.append(_absorb(nc, prev_xt[1][0:1, 0:1]))
        xt_sb = []
        for g in range(2):
            xt_ps = ps_xt.tile([128, 128], F32)
            mm_t = nc.tensor.transpose(
                xt_ps[:], xb[:, 4 * g:4 * g + 4, :], identity[:, :])
            order(mm_t, *pre_abs)
            xts = xt_sbp.tile([128, 128], F32)
            if g == 0:
                nc.vector.tensor_copy(xts[:], xt_ps[:])
            else:
                nc.scalar.copy(xts[:], xt_ps[:])
            xt_sb.append(xts)
        prev_xt = xt_sb
        mm1_abs = [_absorb(nc, xt_sb[0][0:1, 0:1]),
                   _absorb(nc, xt_sb[1][0:1, 0:1]),
                   _absorb(nc, wblk[0:1, 0:1])]

        # s = x @ W: one K=128 matmul per transpose group against the
        # block-diagonal W gives all 4 chunks' s columns (128, 4*116)
        s_ps = ps_s.tile([128, 2, 512], F32)
        for g in range(2):
            mm = nc.tensor.matmul(
                s_ps[:, g, 0:4 * K], xt_sb[g][:], wblk[:],
                start=True, stop=True)
            order(mm, *mm1_abs)

        # e = exp(s) in one ACT op; chunk (g,q) at e_sb[:, g, q, :]
        pstep = s_ps[:].ap[0][0]
        s_view = _ap(s_ps, 0, [[pstep, 128], [512, 2], [K, 4], [1, K]])
        e_sb = epool.tile([128, 2, 4, K], F32)
        nc.scalar.activation(e_sb[:], s_view, AF.Exp)

        # softmax denominators per pixel, then r = 1/sums
        sums = small.tile([128, 2, 4], F32)
        nc.vector.tensor_reduce(sums[:], e_sb[:], axis=mybir.AxisListType.X,
                                op=ALU.add)
        r = small.tile([128, 2, 4], F32)
        nc.vector.reciprocal(r[:], sums[:])

        # x' = [x * r | r]  (per chunk scale by per-pixel 1/sum)
        da_xb = dve_absorb(xb[0:1, 0, 0:1]) if b == 0 else None
        xp = xppool.tile([128, T, D + 1], F32)
        xpstep = xp[:].ap[0][0]
        for g in range(2):
            for q in range(4):
                t = 4 * g + q
                sc = nc.vector.tensor_scalar_mul(
                    xp[:, t, 0:D], xb[:, t, :], r[:, g, q:q + 1])
                if da_xb is not None:
                    order(sc, da_xb)
        # r column via 0*e + r: bit-identical to a copy of r, but reading e
        # makes this last x'-writer carry the exp dependency so the pooling
        # matmuls need only the one DVE semaphore
        e_ps0 = e_sb[:].ap[0][0]
        r_ps0 = r[:].ap[0][0]
        nc.vector.scalar_tensor_tensor(
            out=_ap(xp, D, [[xpstep, 128], [(D + 1) * 4, 2], [D + 1, 4]]),
            in0=_ap(e_sb, 0, [[e_ps0, 128], [4 * K, 2], [K, 4]]),
            scalar=0.0,
            in1=_ap(r, 0, [[r_ps0, 128], [4, 2], [1, 4]]),
            op0=ALU.mult, op1=ALU.add)

        # pooling: [v1.T | asum] = e.T @ x' accumulated over the 8 chunks
        mm2_abs = [_absorb(nc, xp[0:1, T - 1, D:D + 1]),
                   _absorb(nc, e_sb[0:1, 0, 0, 0:1])]
        v_ps = ps_v.tile([K, D + 1], F32)
        for g in range(2):
            for q in range(4):
                t = 4 * g + q
                mm = nc.tensor.matmul(
                    v_ps[:], e_sb[:, g, q, :], xp[:, t, :],
                    start=(t == 0), stop=(t == T - 1),
                )
                order(mm, *mm2_abs)

        # v.T = v1.T + asum * C.T   (fused multiply-add on DVE)
        da_v = dve_absorb(v_ps[0:1, 0:1])
        stt = nc.vector.scalar_tensor_tensor(
            out=vstage[:, b, :], in0=ct_sb[:], scalar=v_ps[:, D:D + 1],
            in1=v_ps[:, 0:D], op0=ALU.mult, op1=ALU.add)
        order(stt, da_v)

        # ss[k] = sum_d v.T[k,d]^2
        sq = small.tile([K, D], F32, tag="sq")
        nc.vector.tensor_mul(sq[:], vstage[:, b, :], vstage[:, b, :])
        nc.vector.tensor_reduce(ss[:, b:b + 1], sq[:],
                                axis=mybir.AxisListType.X, op=ALU.add)

    # ---- phase B: normalization for all batches -------------------------
    # rinv = 1/sqrt(ss+eps) via exp(-0.5*ln(ss+eps))  (single ACT table set)
    lss = small.tile([K, BPC], F32, tag="lss")
    nc.scalar.activation(lss[:], ss[:], AF.Ln, bias=epsb[0:K, 0:1])
    rinv = singles.tile([K, BPC], F32)
    nc.scalar.activation(rinv[:], lss[:], AF.Exp, scale=-0.5)

    # g[b] = sum_k ss*rinv^2 ; column sums via ones-matmul -> (1, BPC)
    da_ri = dve_absorb(rinv[0:1, 0:1])
    t2 = small.tile([K, BPC], F32, tag="t2")
    tm = nc.vector.tensor_mul(t2[:], ss[:], rinv[:])
    order(tm, da_ri)
    nc.vector.tensor_mul(t2[:], t2[:], rinv[:])
    g_ps = ps_b.tile([1, BPC], F32, tag="phb")
    nc.tensor.matmul(g_ps[:], ones_t[0:K, 0:1], t2[:], start=True, stop=True)

    gr = singles.tile([1, BPC], F32)
    nc.scalar.activation(gr[:], g_ps[:], AF.Ln, bias=epsb[0:1, 0:1])
    nc.scalar.activation(gr[:], gr[:], AF.Exp, scale=-0.5)

    # broadcast gr over the 116 partitions, total scale S = rinv * gr
    a_on = _absorb(nc, ones_t[0:1, 0:1])
    a_gr = _absorb(nc, gr[0:1, 0:1])
    grb_ps = ps_b.tile([K, BPC], F32, tag="phb")
    mm_gb = nc.tensor.matmul(grb_ps[:], ones_t[0:1, 0:K], gr[:],
                             start=True, stop=True)
    order(mm_gb, a_on, a_gr)
    da_gb = dve_absorb(grb_ps[0:1, 0:1])
    s_all = singles.tile([K, BPC], F32)
    sm = nc.vector.tensor_mul(s_all[:], rinv[:], grb_ps[:])
    order(sm, da_gb)

    # y.T = v.T * S, transpose back to (D, K), stage, one output DMA
    for b in range(BPC):
        yt = ytp.tile([K, D], F32)
        nc.vector.tensor_scalar_mul(yt[:], vstage[:, b, :], s_all[:, b:b + 1])
        y_ps = ps_b.tile([D, K], F32, tag="phb")
        nc.tensor.transpose(y_ps[:], yt[:], identity[0:K, 0:K])
        nc.vector.tensor_copy(ysb[:, b, :], y_ps[:])

    y_dst = _ap(y_out[:, :], 0, [[K, D], [D * K, BPC], [1, K]])
    nc.sync.dma_start(out=y_dst, in_=ysb[:])


def _split_drain_and_barrier(self, tick_clock, wait_clock):
    """Replacement for TileContext._drain_and_barrier: this walrus build
    accepts at most one sync wait per instruction, so the kernel-tail drain's
    per-proc waits are spread over a chain of one-wait drains."""
    from concourse.vector_clock import ScopedClock

    nc = self.nc
    drain_inst = nc.sync.drain()
    wait_clock.add_sem_waits(
        drain_inst.ins, ScopedClock({None: tick_clock.global_clock}))
    si = drain_inst.ins.sync_info
    if si is not None and len(si.on_wait) > 1:
        waits = list(si.on_wait)
        upd = list(si.on_update)
        drain_inst.ins.sync_info = mybir.SyncInfo(
            on_wait=[waits[0]], on_update=upd)
        for w in waits[1:]:
            d2 = nc.sync.drain()
            d2.ins.sync_info = mybir.SyncInfo(on_wait=[w], on_update=[])

    nc.all_engine_barrier()
    assert self.sems is not None
    popped = nc._tile_sem_poison_stack.pop()
    assert popped is self._sem_poison
    # skip clear_and_free_semaphores: its EVENT_SEMAPHORE_RANGE_CLEAR InstISA
    # fails codegen here, and this kernel is built fresh per NEFF load so the
    # semaphores are never recycled.
    nc.all_engine_barrier()


def build_bass(cut=99):
    import types

    nc = bass.Bass()
    x_in = nc.declare_dram_parameter("x_loc", [BPC, N, D], F32, isOutput=False)
    w_in = nc.declare_dram_parameter("w_in", [D, K], F32, isOutput=False)
    c_in = nc.declare_dram_parameter("c_in", [D, K], F32, isOutput=False)
    y_out = nc.declare_dram_parameter("y_loc", [BPC, D * K], F32, isOutput=True)
    with ExitStack() as ctx:
        tc = ctx.enter_context(tile.TileContext(nc))
        tc._drain_and_barrier = types.MethodType(_split_drain_and_barrier, tc)
        _emit(ctx, tc, y_out, x_in, w_in, c_in)
    return nc


def run(x, W, C, trace=False, tmpdir=None):
    from concourse.bass_utils import run_bass_kernel_spmd

    x = np.ascontiguousarray(x, dtype=np.float32).reshape(B, N, D)
    W = np.ascontiguousarray(W, dtype=np.float32)
    C = np.ascontiguousarray(C, dtype=np.float32)

    nc = build_bass()
    in_maps = [
        {"x_loc": x[c * BPC:(c + 1) * BPC], "w_in": W, "c_in": C}
        for c in range(NCORES)
    ]
    res = run_bass_kernel_spmd(nc, in_maps, list(range(NCORES)), trace=trace,
                               tmpdir=tmpdir)
    y = np.concatenate([res.results[c]["y_loc"] for c in range(NCORES)], axis=0)
    return y.astype(np.float32), res


def kernel(x, W, C):
    y, _ = run(x, W, C, trace=False)
    return y


# revision 48
# speedup vs baseline: 1.1299x; 1.1299x over previous
"""CenterVLAD Trainium2 kernel.

Reference computation (per batch b, with N = H*W = 1024 pixels, D=32, K=116):
    s = x @ W                    # (N, K)
    a = softmax(s, axis=-1)
    v = x.T @ a + (sum_n a) * C  # (D, K)
    v /= sqrt(sum_d v^2 + eps)   # intra-norm over D
    y = v.flatten()
    y /= sqrt(sum y^2 + eps)     # global L2

Sharding: data-parallel over batch B=32 across 8 cores (4 batches/core),
W and C replicated, no collectives.

Per-core layout: pixels-on-partitions.  Pixel n = 8*p + t lives on partition
p, sub-chunk t in 0..7; chunk (g, q) (t = 4g+q) is the 128-pixel set
{8p + 4g + q}.  The PE transposes 4 chunks at a time ((128,[4,32]) ->
(128,128), all at partition base 0 -- this device path rejects matmuls whose
operands sit at a nonzero partition base, so K=32 row-packing via
tile_position is unusable).  s = x @ W then runs as ONE matmul per transpose
group against a block-diagonal zero-masked W (128 x 4*116): the zero rows
mask out the other three chunks, so a full K=128 contraction yields all four
chunks' s columns side by side.  Softmax runs in pixel-partition layout; the
pooling matmul e.T @ [x*r | r] contracts over pixels giving (116, 33) =
[v1.T | asum] so the intra-normalization runs along the free dim; rsqrt is
exp(-0.5*ln(.)) (one ACT table set; ACT Rsqrt/Sqrt are banned/inaccurate).

Toolchain quirks this kernel works around:
  * walrus here accepts at most ONE sync wait per instruction: dead-end 1x1
    "absorber" matmuls/copies make an engine observe foreign semaphores
    first (pinned before their consumers via add_dep_helper, since the
    scheduler would otherwise sink dead-end ops), SBUF pools are sized so
    per-batch tiles are never reused (no WAR deps), and the Tile kernel-tail
    drain is split into a chain of one-wait drains.
  * EVENT_SEMAPHORE_RANGE_CLEAR and tensor_tensor_reduce fail codegen; the
    sem clear is skipped (fresh NEFF per load) and square+reduce is two ops.
"""

import numpy as np
from contextlib import ExitStack

import concourse.bass as bass
import concourse.tile as tile
from concourse import mybir, masks
from concourse.tile import add_dep_helper

F32 = mybir.dt.float32
BF16 = mybir.dt.bfloat16
AF = mybir.ActivationFunctionType
ALU = mybir.AluOpType

B = 32          # total batches
N = 1024        # H*W pixels per batch
D = 32          # channels
K = 116         # clusters
NCORES = 8
BPC = B // NCORES   # batches per core
T = 8               # pixel sub-chunks per batch (each chunk = 128 pixels)
EPS = 1e-12


def _ap(t, offs_el, dims):
    """Manual AP over tile/dram handle `t`: dims = [[step, count], ...] in
    elements, first dim = partition."""
    base = t[:] if not isinstance(t, bass.AP) else t
    return bass.AP(tensor=base.tensor, offset=base.offset + offs_el, ap=dims)


_JUNK = []


def _absorb(nc, ap):
    """1x1 dummy matmul on the PE whose only role is to make the PE observe
    `ap`'s producer semaphore (walrus allows one sync wait per instruction).
    Writes a dedicated write-only PSUM bank; matmul-after-matmul ordering is
    program order so no extra semaphore appears."""
    return nc.tensor.matmul(_JUNK[0][0:1, 0:1], ap, ap, start=True, stop=True,
                            tile_position=(0, 0))


def _emit(ctx, tc, y_out, x_in, w_in, c_in):
    nc = tc.nc

    singles = ctx.enter_context(tc.tile_pool(name="singles", bufs=1))
    xt_sbp = ctx.enter_context(tc.tile_pool(name="xt_sbp", bufs=8))
    epool = ctx.enter_context(tc.tile_pool(name="epool", bufs=4))
    small = ctx.enter_context(tc.tile_pool(name="small", bufs=8))
    xppool = ctx.enter_context(tc.tile_pool(name="xppool", bufs=4))
    ytp = ctx.enter_context(tc.tile_pool(name="ytp", bufs=4))

    ps_xt = ctx.enter_context(tc.tile_pool(name="ps_xt", bufs=2, space="PSUM"))
    ps_s = ctx.enter_context(tc.tile_pool(name="ps_s", bufs=1, space="PSUM"))
    ps_v = ctx.enter_context(tc.tile_pool(name="ps_v", bufs=1, space="PSUM"))
    ps_b = ctx.enter_context(tc.tile_pool(name="ps_b", bufs=2, space="PSUM"))
    ps_j = ctx.enter_context(tc.tile_pool(name="ps_j", bufs=1, space="PSUM"))
    _JUNK.clear()
    junk_ps = ps_j.tile([1, 1], F32, tag="junk")
    _JUNK.append(junk_ps)
    djunk = singles.tile([1, 64], F32)
    dj_i = [0]

    def dve_absorb(ap):
        # absorber for the DVE; each writes its own djunk element so
        # consecutive absorbers don't WAW-chain each other
        i = dj_i[0] % 64
        dj_i[0] += 1
        return nc.vector.tensor_copy(djunk[0:1, i:i + 1], ap)

    def order(consumer, *absorbers):
        # absorbers are dead-end ops; the priority scheduler would otherwise
        # sink them below the very instructions they must precede
        for a in absorbers:
            add_dep_helper(consumer.ins, a.ins, reason="absorber ordering")

    # ---- constants -------------------------------------------------------
    identity = singles.tile([128, 128], F32)
    masks.make_identity(nc, identity[:])

    ones_t = singles.tile([128, K], F32)
    nc.vector.memset(ones_t[:], 1.0)

    epsb = singles.tile([128, 1], F32)
    nc.vector.memset(epsb[:], EPS)

    w_sb = singles.tile([D, K], F32)
    nc.sync.dma_start(out=w_sb[:], in_=w_in[:, :])
    c_sb = singles.tile([D, K], F32)
    nc.sync.dma_start(out=c_sb[:], in_=c_in[:, :])

    # PE pre-observes the constant producers so later matmuls carry at most
    # one sync wait each.
    a_id = _absorb(nc, identity[0:1, 0:1])
    a_w = _absorb(nc, w_sb[0:1, 0:1])

    # replicate W onto the 4 partition groups (column packing works on this
    # device path), then scatter into the block-diagonal zero-masked W
    w4_ps = ps_b.tile([128, K], F32, tag="phb")
    for q in range(4):
        mm_w4 = nc.tensor.matmul(
            w4_ps[32 * q:32 * q + 32, :], identity[0:D, 0:D], w_sb[:],
            start=True, stop=True, tile_position=(0, 32 * q))
        order(mm_w4, a_id, a_w)
    wblk = singles.tile([128, 4 * K], F32)
    nc.vector.memset(wblk[:], 0.0)
    ajunk = singles.tile([1, 8], F32)
    ab1 = nc.scalar.copy(ajunk[0:1, 0:1], wblk[0:1, 0:1])        # DVE memset
    ab2 = nc.scalar.copy(ajunk[0:1, 1:2], w4_ps[96:97, 0:1])     # PE matmuls
    for q in range(4):
        cp = nc.scalar.copy(wblk[32 * q:32 * q + 32, K * q:K * q + K],
                            w4_ps[32 * q:32 * q + 32, :])
        order(cp, ab1, ab2)
    # bf16 hi/lo split of W so mm1 runs as three 1-pass bf16 matmuls with
    # near-fp32 accuracy (the dropped lo*lo term is ~2^-18)
    wh = singles.tile([128, 4 * K], BF16)
    nc.scalar.copy(wh[:], wblk[:])
    wl = singles.tile([128, 4 * K], BF16)
    da_wh = dve_absorb(wh[0:1, 0:1])
    wlop = nc.vector.scalar_tensor_tensor(out=wl[:], in0=wblk[:], scalar=1.0,
                                          in1=wh[:], op0=ALU.mult,
                                          op1=ALU.subtract)
    order(wlop, da_wh)

    ct_ps = ps_b.tile([K, D], F32, tag="phb")
    mm_ct = nc.tensor.transpose(ct_ps[:], c_sb[:], identity[0:D, 0:D])
    order(mm_ct, a_id)
    ct_sb = singles.tile([K, D], F32)
    nc.vector.tensor_copy(ct_sb[:], ct_ps[:])

    # ---- input load: one 512 KiB DMA for all 4 batches (fewer DMA lanes
    # keeps the kernel-tail drain's wait list small) -----------------------
    xall = singles.tile([128, BPC, T, D], F32)
    nc.sync.dma_start(
        out=xall[:],
        in_=x_in.rearrange("b (p t) d -> p b t d", t=T))
    xx = [xall[:, b] for b in range(BPC)]

    # ---- per-core staging for the normalization phase --------------------
    vstage = singles.tile([K, BPC, D], F32)    # [v1.T + asum*C.T] per batch
    ss = singles.tile([K, BPC], F32)           # intra-norm sum of squares
    ysb = singles.tile([D, BPC, K], F32)       # final output staging

    # ---- phase A: per batch ---------------------------------------------
    prev_cast = None
    for b in range(BPC):
        xb = xx[b]

        # transpose 4 chunks at a time: (128 pix, [t4,d32]) -> (128, 128 pix)
        # both groups into one single-bank PSUM tile
        xt_ps = ps_xt.tile([128, 2, 128], F32)
        for g in range(2):
            mm_t = nc.tensor.transpose(
                xt_ps[:, g, :], xb[:, 4 * g:4 * g + 4, :], identity[:, :])
            if b == 0 and g == 0:
                mm_xb = _absorb(nc, xb[0:1, 0, 0:1])     # input-DMA lane
                order(mm_t, mm_xb)
            if prev_cast is not None:
                order(mm_t, *prev_cast)

        # bf16 hi/lo split of the transposed x, straight from PSUM; both
        # casts on the DVE so exp keeps a single (PE) wait
        da_t = dve_absorb(xt_ps[0:1, 0, 0:1])
        xh = xt_sbp.tile([128, 2, 128], BF16, tag="xh")
        c1 = nc.vector.tensor_copy(xh[:], xt_ps[:])
        order(c1, da_t)
        xl = xt_sbp.tile([128, 2, 128], BF16, tag="xl")
        nc.vector.scalar_tensor_tensor(out=xl[:], in0=xt_ps[:], scalar=1.0,
                                       in1=xh[:], op0=ALU.mult,
                                       op1=ALU.subtract)
        a_xl = _absorb(nc, xl[0:1, 0, 0:1])    # DVE, covers xh too
        prev_cast = (a_xl,)

        # s = x @ W: per transpose group, three accumulating bf16 matmuls
        # against the block-diagonal hi/lo W give all 4 chunks' s columns
        s_ps = ps_s.tile([128, 2, 512], F32)
        e_sb = epool.tile([128, 2, 4, K], F32)
        sums = small.tile([128, 2, 4], F32)
        pstep = s_ps[:].ap[0][0]
        for g in range(2):
            for i, (lt, rt) in enumerate(((xh, wh), (xl, wh), (xh, wl))):
                mm = nc.tensor.matmul(
                    s_ps[:, g, 0:4 * K], lt[:, g, :], rt[:],
                    start=(i == 0), stop=(i == 2))
                order(mm, a_xl)
            # e = exp(s); one ACT + one DVE reduce per group keeps the
            # critical chain short
            s_view = _ap(s_ps, 512 * g, [[pstep, 128], [K, 4], [1, K]])
            nc.scalar.activation(e_sb[:, g], s_view, AF.Exp)
            nc.vector.tensor_reduce(sums[:, g], e_sb[:, g],
                                    axis=mybir.AxisListType.X, op=ALU.add)

        r = small.tile([128, 2, 4], F32)
        nc.vector.reciprocal(r[:], sums[:])

        # x' = [x * r | r]: one broadcast multiply (r broadcast over d via a
        # 0-step AP), then the r column via 0*e + r (reads e so the pooling
        # matmuls need only one DVE semaphore)
        xp = xppool.tile([128, T, D + 1], F32)
        xpstep = xp[:].ap[0][0]
        xastep = xall[:].ap[0][0]
        r_ps0 = r[:].ap[0][0]
        e_ps0 = e_sb[:].ap[0][0]
        if b == 0:
            da_xa = dve_absorb(xall[0:1, 0, 0, 0:1])
        sc_tt = nc.vector.tensor_mul(
            _ap(xp, 0, [[xpstep, 128], [(D + 1) * 4, 2], [D + 1, 4], [1, D]]),
            _ap(xall, b * T * D, [[xastep, 128], [4 * D, 2], [D, 4], [1, D]]),
            _ap(r, 0, [[r_ps0, 128], [4, 2], [1, 4], [0, D]]))
        if b == 0:
            order(sc_tt, da_xa)
        nc.vector.scalar_tensor_tensor(
            out=_ap(xp, D, [[xpstep, 128], [(D + 1) * 4, 2], [D + 1, 4]]),
            in0=_ap(e_sb, 0, [[e_ps0, 128], [4 * K, 2], [K, 4]]),
            scalar=0.0,
            in1=_ap(r, 0, [[r_ps0, 128], [4, 2], [1, 4]]),
            op0=ALU.mult, op1=ALU.add)

        # pooling: [v1.T | asum] = e.T @ x' accumulated over the 8 chunks
        mm2_abs = [_absorb(nc, xp[0:1, T - 1, D:D + 1]),
                   _absorb(nc, e_sb[0:1, 0, 0, 0:1])]
        v_ps = ps_v.tile([K, D + 1], F32)
        for g in range(2):
            for q in range(4):
                t = 4 * g + q
                mm = nc.tensor.matmul(
                    v_ps[:], e_sb[:, g, q, :], xp[:, t, :],
                    start=(t == 0), stop=(t == T - 1),
                )
                order(mm, *mm2_abs)

        # v.T = v1.T + asum * C.T   (fused multiply-add on DVE)
        da_v = dve_absorb(v_ps[0:1, 0:1])
        stt = nc.vector.scalar_tensor_tensor(
            out=vstage[:, b, :], in0=ct_sb[:], scalar=v_ps[:, D:D + 1],
            in1=v_ps[:, 0:D], op0=ALU.mult, op1=ALU.add)
        order(stt, da_v)

        # ss[k] = sum_d v.T[k,d]^2
        sq = small.tile([K, D], F32, tag="sq")
        nc.vector.tensor_mul(sq[:], vstage[:, b, :], vstage[:, b, :])
        nc.vector.tensor_reduce(ss[:, b:b + 1], sq[:],
                                axis=mybir.AxisListType.X, op=ALU.add)

    # ---- phase B: normalization for all batches -------------------------
    # rinv = 1/sqrt(ss+eps) via exp(-0.5*ln(ss+eps))  (single ACT table set)
    lss = small.tile([K, BPC], F32, tag="lss")
    nc.scalar.activation(lss[:], ss[:], AF.Ln, bias=epsb[0:K, 0:1])
    rinv = singles.tile([K, BPC], F32)
    nc.scalar.activation(rinv[:], lss[:], AF.Exp, scale=-0.5)

    # g[b] = sum_k ss*rinv^2 ; column sums via ones-matmul -> (1, BPC)
    da_ri = dve_absorb(rinv[0:1, 0:1])
    t2 = small.tile([K, BPC], F32, tag="t2")
    tm = nc.vector.tensor_mul(t2[:], ss[:], rinv[:])
    order(tm, da_ri)
    nc.vector.tensor_mul(t2[:], t2[:], rinv[:])
    g_ps = ps_b.tile([1, BPC], F32, tag="phb")
    nc.tensor.matmul(g_ps[:], ones_t[0:K, 0:1], t2[:], start=True, stop=True)

    gr = singles.tile([1, BPC], F32)
    nc.scalar.activation(gr[:], g_ps[:], AF.Ln, bias=epsb[0:1, 0:1])
    nc.scalar.activation(gr[:], gr[:], AF.Exp, scale=-0.5)

    # broadcast gr over the 116 partitions, total scale S = rinv * gr
    a_on = _absorb(nc, ones_t[0:1, 0:1])
    a_gr = _absorb(nc, gr[0:1, 0:1])
    grb_ps = ps_b.tile([K, BPC], F32, tag="phb")
    mm_gb = nc.tensor.matmul(grb_ps[:], ones_t[0:1, 0:K], gr[:],
                             start=True, stop=True)
    order(mm_gb, a_on, a_gr)
    da_gb = dve_absorb(grb_ps[0:1, 0:1])
    s_all = singles.tile([K, BPC], F32)
    sm = nc.vector.tensor_mul(s_all[:], rinv[:], grb_ps[:])
    order(sm, da_gb)

    # y.T = v.T * S, transpose back to (D, K), stage, one output DMA
    for b in range(BPC):
        yt = ytp.tile([K, D], F32)
        nc.vector.tensor_scalar_mul(yt[:], vstage[:, b, :], s_all[:, b:b + 1])
        y_ps = ps_b.tile([D, K], F32, tag="phb")
        nc.tensor.transpose(y_ps[:], yt[:], identity[0:K, 0:K])
        nc.vector.tensor_copy(ysb[:, b, :], y_ps[:])

    y_dst = _ap(y_out[:, :], 0, [[K, D], [D * K, BPC], [1, K]])
    nc.sync.dma_start(out=y_dst, in_=ysb[:])


def _split_drain_and_barrier(self, tick_clock, wait_clock):
    """Replacement for TileContext._drain_and_barrier: this walrus build
    accepts at most one sync wait per instruction, so the kernel-tail drain's
    per-proc waits are spread over a chain of one-wait drains."""
    from concourse.vector_clock import ScopedClock

    nc = self.nc
    drain_inst = nc.sync.drain()
    wait_clock.add_sem_waits(
        drain_inst.ins, ScopedClock({None: tick_clock.global_clock}))
    si = drain_inst.ins.sync_info
    if si is not None and len(si.on_wait) > 1:
        waits = list(si.on_wait)
        upd = list(si.on_update)
        drain_inst.ins.sync_info = mybir.SyncInfo(
            on_wait=[waits[0]], on_update=upd)
        for w in waits[1:]:
            d2 = nc.sync.drain()
            d2.ins.sync_info = mybir.SyncInfo(on_wait=[w], on_update=[])

    nc.all_engine_barrier()
    assert self.sems is not None
    popped = nc._tile_sem_poison_stack.pop()
    assert popped is self._sem_poison
    # skip clear_and_free_semaphores: its EVENT_SEMAPHORE_RANGE_CLEAR InstISA
    # fails codegen here, and this kernel is built fresh per NEFF load so the
    # semaphores are never recycled.
    nc.all_engine_barrier()


def build_bass(cut=99):
    import types

    nc = bass.Bass()
    x_in = nc.declare_dram_parameter("x_loc", [BPC, N, D], F32, isOutput=False)
    w_in = nc.declare_dram_parameter("w_in", [D, K], F32, isOutput=False)
    c_in = nc.declare_dram_parameter("c_in", [D, K], F32, isOutput=False)
    y_out = nc.declare_dram_parameter("y_loc", [BPC, D * K], F32, isOutput=True)
    with ExitStack() as ctx:
        tc = ctx.enter_context(tile.TileContext(nc))
        tc._drain_and_barrier = types.MethodType(_split_drain_and_barrier, tc)
        _emit(ctx, tc, y_out, x_in, w_in, c_in)
    # strip same-engine self-waits from multi-wait instructions: the engines
    # dispatch in FIFO order and DVE/ACT drain between ops, so a self-wait
    # whose target precedes in the same stream guards only pseudo-hazards
    # (PSUM bank read-read serialization); walrus allows one wait only.
    eng_name = {mybir.EngineType.Activation: "Activation",
                mybir.EngineType.PE: "PE",
                mybir.EngineType.DVE: "DVE",
                mybir.EngineType.Pool: "Pool",
                mybir.EngineType.SP: "SP"}
    for name, inst in nc.inst_map.items():
        si = inst.sync_info
        if si is None or len(si.on_wait) <= 1:
            continue
        en = eng_name.get(getattr(inst, "engine", None))
        if en is None:
            continue
        keep = [w for w in si.on_wait if not w.ant_name.startswith(en + "_")]
        if 0 < len(keep) < len(si.on_wait):
            inst.sync_info = mybir.SyncInfo(on_wait=keep,
                                            on_update=list(si.on_update))
    return nc


def run(x, W, C, trace=False, tmpdir=None):
    from concourse.bass_utils import run_bass_kernel_spmd

    x = np.ascontiguousarray(x, dtype=np.float32).reshape(B, N, D)
    W = np.ascontiguousarray(W, dtype=np.float32)
    C = np.ascontiguousarray(C, dtype=np.float32)

    nc = build_bass()
    in_maps = [
        {"x_loc": x[c * BPC:(c + 1) * BPC], "w_in": W, "c_in": C}
        for c in range(NCORES)
    ]
    res = run_bass_kernel_spmd(nc, in_maps, list(range(NCORES)), trace=trace,
                               tmpdir=tmpdir)
    y = np.concatenate([res.results[c]["y_loc"] for c in range(NCORES)], axis=0)
    return y.astype(np.float32), res


def kernel(x, W, C):
    y, _ = run(x, W, C, trace=False)
    return y


# revision 50
# speedup vs baseline: 1.3598x; 1.2035x over previous
"""CenterVLAD Trainium2 kernel.

Reference computation (per batch b, with N = H*W = 1024 pixels, D=32, K=116):
    s = x @ W                    # (N, K)
    a = softmax(s, axis=-1)
    v = x.T @ a + (sum_n a) * C  # (D, K)
    v /= sqrt(sum_d v^2 + eps)   # intra-norm over D
    y = v.flatten()
    y /= sqrt(sum y^2 + eps)     # global L2

Sharding: data-parallel over batch B=32 across 8 cores (4 batches/core),
W and C replicated, no collectives.

Per-core layout: pixels-on-partitions.  Pixel n = 8*p + t lives on partition
p, sub-chunk t in 0..7; chunk (g, q) (t = 4g+q) is the 128-pixel set
{8p + 4g + q}.  The PE transposes 4 chunks at a time ((128,[4,32]) ->
(128,128), all at partition base 0 -- this device path rejects matmuls whose
operands sit at a nonzero partition base, so K=32 row-packing via
tile_position is unusable).  s = x @ W then runs as ONE matmul per transpose
group against a block-diagonal zero-masked W (128 x 4*116): the zero rows
mask out the other three chunks, so a full K=128 contraction yields all four
chunks' s columns side by side.  Softmax runs in pixel-partition layout; the
pooling matmul e.T @ [x*r | r] contracts over pixels giving (116, 33) =
[v1.T | asum] so the intra-normalization runs along the free dim; rsqrt is
exp(-0.5*ln(.)) (one ACT table set; ACT Rsqrt/Sqrt are banned/inaccurate).

Toolchain quirks this kernel works around:
  * walrus here accepts at most ONE sync wait per instruction: dead-end 1x1
    "absorber" matmuls/copies make an engine observe foreign semaphores
    first (pinned before their consumers via add_dep_helper, since the
    scheduler would otherwise sink dead-end ops), SBUF pools are sized so
    per-batch tiles are never reused (no WAR deps), and the Tile kernel-tail
    drain is split into a chain of one-wait drains.
  * EVENT_SEMAPHORE_RANGE_CLEAR and tensor_tensor_reduce fail codegen; the
    sem clear is skipped (fresh NEFF per load) and square+reduce is two ops.
"""

import numpy as np
from contextlib import ExitStack

import concourse.bass as bass
import concourse.tile as tile
from concourse import mybir, masks
from concourse.tile import add_dep_helper

F32 = mybir.dt.float32
BF16 = mybir.dt.bfloat16
AF = mybir.ActivationFunctionType
ALU = mybir.AluOpType

B = 32          # total batches
N = 1024        # H*W pixels per batch
D = 32          # channels
K = 116         # clusters
NCORES = 8
BPC = B // NCORES   # batches per core
T = 8               # pixel sub-chunks per batch (each chunk = 128 pixels)
EPS = 1e-12


def _ap(t, offs_el, dims):
    """Manual AP over tile/dram handle `t`: dims = [[step, count], ...] in
    elements, first dim = partition."""
    base = t[:] if not isinstance(t, bass.AP) else t
    return bass.AP(tensor=base.tensor, offset=base.offset + offs_el, ap=dims)


_JUNK = []


def _absorb(nc, ap):
    """1x1 dummy matmul on the PE whose only role is to make the PE observe
    `ap`'s producer semaphore (walrus allows one sync wait per instruction).
    Writes a dedicated write-only PSUM bank; matmul-after-matmul ordering is
    program order so no extra semaphore appears."""
    if ap.dtype == F32:
        ap = ap.bitcast(BF16)[0:1, 0:1]
    return nc.tensor.matmul(_JUNK[0][0:1, 0:1], ap, ap, start=True, stop=True,
                            tile_position=(0, 0))


def _emit(ctx, tc, y_out, x_in, w_in, c_in):
    nc = tc.nc

    singles = ctx.enter_context(tc.tile_pool(name="singles", bufs=1))
    xt_sbp = ctx.enter_context(tc.tile_pool(name="xt_sbp", bufs=8))
    epool = ctx.enter_context(tc.tile_pool(name="epool", bufs=4))
    small = ctx.enter_context(tc.tile_pool(name="small", bufs=8))
    xppool = ctx.enter_context(tc.tile_pool(name="xppool", bufs=4))
    ytp = ctx.enter_context(tc.tile_pool(name="ytp", bufs=4))

    ps_xt = ctx.enter_context(tc.tile_pool(name="ps_xt", bufs=2, space="PSUM"))
    ps_s = ctx.enter_context(tc.tile_pool(name="ps_s", bufs=1, space="PSUM"))
    ps_v = ctx.enter_context(tc.tile_pool(name="ps_v", bufs=1, space="PSUM"))
    ps_b = ctx.enter_context(tc.tile_pool(name="ps_b", bufs=2, space="PSUM"))
    ps_j = ctx.enter_context(tc.tile_pool(name="ps_j", bufs=1, space="PSUM"))
    _JUNK.clear()
    junk_ps = ps_j.tile([1, 1], F32, tag="junk")
    _JUNK.append(junk_ps)
    djunk = singles.tile([1, 64], F32)
    dj_i = [0]

    def dve_absorb(ap):
        # absorber for the DVE; each writes its own djunk element so
        # consecutive absorbers don't WAW-chain each other
        i = dj_i[0] % 64
        dj_i[0] += 1
        return nc.vector.tensor_copy(djunk[0:1, i:i + 1], ap)

    def order(consumer, *absorbers):
        # absorbers are dead-end ops; the priority scheduler would otherwise
        # sink them below the very instructions they must precede
        for a in absorbers:
            add_dep_helper(consumer.ins, a.ins, reason="absorber ordering")

    # ---- constants -------------------------------------------------------
    identity = singles.tile([128, 128], F32)
    masks.make_identity(nc, identity[:])

    ones_t = singles.tile([128, K], F32)
    nc.vector.memset(ones_t[:], 1.0)

    epsb = singles.tile([128, 1], F32)
    nc.vector.memset(epsb[:], EPS)

    w_sb = singles.tile([D, K], F32)
    nc.sync.dma_start(out=w_sb[:], in_=w_in[:, :])
    c_sb = singles.tile([D, K], F32)
    nc.sync.dma_start(out=c_sb[:], in_=c_in[:, :])

    # PE pre-observes the constant producers so later matmuls carry at most
    # one sync wait each.
    a_id = _absorb(nc, identity[0:1, 0:1])
    a_w = _absorb(nc, w_sb[0:1, 0:1])

    # replicate W onto the 4 partition groups (column packing works on this
    # device path), then scatter into the block-diagonal zero-masked W
    w4_ps = ps_b.tile([128, K], F32, tag="phb")
    for q in range(4):
        mm_w4 = nc.tensor.matmul(
            w4_ps[32 * q:32 * q + 32, :], identity[0:D, 0:D], w_sb[:],
            start=True, stop=True, tile_position=(0, 32 * q))
        order(mm_w4, a_id, a_w)
    wblk = singles.tile([128, 4 * K], F32)
    nc.vector.memset(wblk[:], 0.0)
    ajunk = singles.tile([1, 8], F32)
    ab1 = nc.scalar.copy(ajunk[0:1, 0:1], wblk[0:1, 0:1])        # DVE memset
    ab2 = nc.scalar.copy(ajunk[0:1, 1:2], w4_ps[96:97, 0:1])     # PE matmuls
    for q in range(4):
        cp = nc.scalar.copy(wblk[32 * q:32 * q + 32, K * q:K * q + K],
                            w4_ps[32 * q:32 * q + 32, :])
        order(cp, ab1, ab2)
    # bf16 hi/lo split of W so mm1 runs as three 1-pass bf16 matmuls with
    # near-fp32 accuracy (the dropped lo*lo term is ~2^-18)
    wh = singles.tile([128, 4 * K], BF16)
    nc.scalar.copy(wh[:], wblk[:])
    wl = singles.tile([128, 4 * K], BF16)
    da_wh = dve_absorb(wh[0:1, 0:1])
    wlop = nc.vector.scalar_tensor_tensor(out=wl[:], in0=wblk[:], scalar=1.0,
                                          in1=wh[:], op0=ALU.mult,
                                          op1=ALU.subtract)
    order(wlop, da_wh)

    ct_ps = ps_b.tile([K, D], F32, tag="phb")
    mm_ct = nc.tensor.transpose(ct_ps[:], c_sb[:], identity[0:D, 0:D])
    order(mm_ct, a_id)
    ct_sb = singles.tile([K, D], F32)
    nc.vector.tensor_copy(ct_sb[:], ct_ps[:])

    # ---- input load: one 512 KiB DMA for all 4 batches (fewer DMA lanes
    # keeps the kernel-tail drain's wait list small) -----------------------
    xall = singles.tile([128, BPC, T, D], F32)
    nc.sync.dma_start(
        out=xall[:],
        in_=x_in.rearrange("b (p t) d -> p b t d", t=T))
    xx = [xall[:, b] for b in range(BPC)]

    # ---- per-core staging for the normalization phase --------------------
    vstage = singles.tile([K, BPC, D], F32)    # [v1.T + asum*C.T] per batch
    ss = singles.tile([K, BPC], F32)           # intra-norm sum of squares
    ysb = singles.tile([D, BPC, K], F32)       # final output staging

    # ---- phase A: per batch ---------------------------------------------
    prev_cast = None
    for b in range(BPC):
        xb = xx[b]

        # transpose 4 chunks at a time: (128 pix, [t4,d32]) -> (128, 128 pix)
        # both groups into one single-bank PSUM tile
        xt_ps = ps_xt.tile([128, 2, 128], F32)
        for g in range(2):
            mm_t = nc.tensor.transpose(
                xt_ps[:, g, :], xb[:, 4 * g:4 * g + 4, :], identity[:, :])
            if b == 0 and g == 0:
                mm_xb = _absorb(nc, xb[0:1, 0, 0:1])     # input-DMA lane
                order(mm_t, mm_xb)
            if prev_cast is not None:
                order(mm_t, *prev_cast)

        # bf16 hi/lo split of the transposed x, straight from PSUM; both
        # casts on the DVE so exp keeps a single (PE) wait
        da_t = dve_absorb(xt_ps[0:1, 0, 0:1])
        xh = xt_sbp.tile([128, 2, 128], BF16, tag="xh")
        c1 = nc.vector.tensor_copy(xh[:], xt_ps[:])
        order(c1, da_t)
        xl = xt_sbp.tile([128, 2, 128], BF16, tag="xl")
        nc.vector.scalar_tensor_tensor(out=xl[:], in0=xt_ps[:], scalar=1.0,
                                       in1=xh[:], op0=ALU.mult,
                                       op1=ALU.subtract)
        a_xl = _absorb(nc, xl[0:1, 0, 0:1])    # DVE, covers xh too
        prev_cast = (a_xl,)

        # s = x @ W: per transpose group, three accumulating bf16 matmuls
        # against the block-diagonal hi/lo W give all 4 chunks' s columns
        s_ps = ps_s.tile([128, 2, 512], F32)
        e_sb = epool.tile([128, 2, 4, K], BF16)
        sums = small.tile([128, 2, 4], F32)
        pstep = s_ps[:].ap[0][0]
        for g in range(2):
            for i, (lt, rt) in enumerate(((xh, wh), (xl, wh), (xh, wl))):
                mm = nc.tensor.matmul(
                    s_ps[:, g, 0:4 * K], lt[:, g, :], rt[:],
                    start=(i == 0), stop=(i == 2))
                order(mm, a_xl)
            # e = exp(s); one ACT + one DVE reduce per group keeps the
            # critical chain short
            s_view = _ap(s_ps, 512 * g, [[pstep, 128], [K, 4], [1, K]])
            nc.scalar.activation(e_sb[:, g], s_view, AF.Exp)
            nc.vector.tensor_reduce(sums[:, g], e_sb[:, g],
                                    axis=mybir.AxisListType.X, op=ALU.add)

        r = small.tile([128, 2, 4], F32)
        nc.vector.reciprocal(r[:], sums[:])

        # x' = [x * r | r]: one broadcast multiply (r broadcast over d via a
        # 0-step AP), then the r column via 0*e + r (reads e so the pooling
        # matmuls need only one DVE semaphore)
        xp = xppool.tile([128, T, D + 1], BF16)
        xpstep = xp[:].ap[0][0]
        xastep = xall[:].ap[0][0]
        r_ps0 = r[:].ap[0][0]
        e_ps0 = e_sb[:].ap[0][0]
        if b == 0:
            da_xa = dve_absorb(xall[0:1, 0, 0, 0:1])
        sc_tt = nc.vector.tensor_mul(
            _ap(xp, 0, [[xpstep, 128], [(D + 1) * 4, 2], [D + 1, 4], [1, D]]),
            _ap(xall, b * T * D, [[xastep, 128], [4 * D, 2], [D, 4], [1, D]]),
            _ap(r, 0, [[r_ps0, 128], [4, 2], [1, 4], [0, D]]))
        if b == 0:
            order(sc_tt, da_xa)
        nc.vector.scalar_tensor_tensor(
            out=_ap(xp, D, [[xpstep, 128], [(D + 1) * 4, 2], [D + 1, 4]]),
            in0=_ap(e_sb, 0, [[e_ps0, 128], [4 * K, 2], [K, 4]]),
            scalar=0.0,
            in1=_ap(r, 0, [[r_ps0, 128], [4, 2], [1, 4]]),
            op0=ALU.mult, op1=ALU.add)

        # pooling: [v1.T | asum] = e.T @ x' accumulated over the 8 chunks
        mm2_abs = [_absorb(nc, xp[0:1, T - 1, D:D + 1]),
                   _absorb(nc, e_sb[0:1, 0, 0, 0:1])]
        v_ps = ps_v.tile([K, D + 1], F32)
        for g in range(2):
            for q in range(4):
                t = 4 * g + q
                mm = nc.tensor.matmul(
                    v_ps[:], e_sb[:, g, q, :], xp[:, t, :],
                    start=(t == 0), stop=(t == T - 1),
                )
                order(mm, *mm2_abs)

        # v.T = v1.T + asum * C.T   (fused multiply-add on DVE)
        da_v = dve_absorb(v_ps[0:1, 0:1])
        stt = nc.vector.scalar_tensor_tensor(
            out=vstage[:, b, :], in0=ct_sb[:], scalar=v_ps[:, D:D + 1],
            in1=v_ps[:, 0:D], op0=ALU.mult, op1=ALU.add)
        order(stt, da_v)

        # ss[k] = sum_d v.T[k,d]^2
        sq = small.tile([K, D], F32, tag="sq")
        nc.vector.tensor_mul(sq[:], vstage[:, b, :], vstage[:, b, :])
        nc.vector.tensor_reduce(ss[:, b:b + 1], sq[:],
                                axis=mybir.AxisListType.X, op=ALU.add)

    # ---- phase B: normalization for all batches -------------------------
    # rinv = 1/sqrt(ss+eps) via exp(-0.5*ln(ss+eps))  (single ACT table set)
    lss = small.tile([K, BPC], F32, tag="lss")
    nc.scalar.activation(lss[:], ss[:], AF.Ln, bias=epsb[0:K, 0:1])
    rinv = singles.tile([K, BPC], F32)
    nc.scalar.activation(rinv[:], lss[:], AF.Exp, scale=-0.5)

    # g[b] = sum_k ss*rinv^2 ; column sums via ones-matmul -> (1, BPC)
    da_ri = dve_absorb(rinv[0:1, 0:1])
    t2 = small.tile([K, BPC], F32, tag="t2")
    tm = nc.vector.tensor_mul(t2[:], ss[:], rinv[:])
    order(tm, da_ri)
    nc.vector.tensor_mul(t2[:], t2[:], rinv[:])
    g_ps = ps_b.tile([1, BPC], F32, tag="phb")
    nc.tensor.matmul(g_ps[:], ones_t[0:K, 0:1], t2[:], start=True, stop=True)

    gr = singles.tile([1, BPC], F32)
    nc.scalar.activation(gr[:], g_ps[:], AF.Ln, bias=epsb[0:1, 0:1])
    nc.scalar.activation(gr[:], gr[:], AF.Exp, scale=-0.5)

    # broadcast gr over the 116 partitions, total scale S = rinv * gr
    a_on = _absorb(nc, ones_t[0:1, 0:1])
    a_gr = _absorb(nc, gr[0:1, 0:1])
    grb_ps = ps_b.tile([K, BPC], F32, tag="phb")
    mm_gb = nc.tensor.matmul(grb_ps[:], ones_t[0:1, 0:K], gr[:],
                             start=True, stop=True)
    order(mm_gb, a_on, a_gr)
    da_gb = dve_absorb(grb_ps[0:1, 0:1])
    s_all = singles.tile([K, BPC], F32)
    sm = nc.vector.tensor_mul(s_all[:], rinv[:], grb_ps[:])
    order(sm, da_gb)

    # y.T = v.T * S, transpose back to (D, K), stage, one output DMA
    for b in range(BPC):
        yt = ytp.tile([K, D], F32)
        nc.vector.tensor_scalar_mul(yt[:], vstage[:, b, :], s_all[:, b:b + 1])
        y_ps = ps_b.tile([D, K], F32, tag="phb")
        nc.tensor.transpose(y_ps[:], yt[:], identity[0:K, 0:K])
        nc.vector.tensor_copy(ysb[:, b, :], y_ps[:])

    y_dst = _ap(y_out[:, :], 0, [[K, D], [D * K, BPC], [1, K]])
    nc.sync.dma_start(out=y_dst, in_=ysb[:])


def _split_drain_and_barrier(self, tick_clock, wait_clock):
    """Replacement for TileContext._drain_and_barrier: this walrus build
    accepts at most one sync wait per instruction, so the kernel-tail drain's
    per-proc waits are spread over a chain of one-wait drains."""
    from concourse.vector_clock import ScopedClock

    nc = self.nc
    drain_inst = nc.sync.drain()
    wait_clock.add_sem_waits(
        drain_inst.ins, ScopedClock({None: tick_clock.global_clock}))
    si = drain_inst.ins.sync_info
    if si is not None and len(si.on_wait) > 1:
        waits = list(si.on_wait)
        upd = list(si.on_update)
        drain_inst.ins.sync_info = mybir.SyncInfo(
            on_wait=[waits[0]], on_update=upd)
        for w in waits[1:]:
            d2 = nc.sync.drain()
            d2.ins.sync_info = mybir.SyncInfo(on_wait=[w], on_update=[])

    nc.all_engine_barrier()
    assert self.sems is not None
    popped = nc._tile_sem_poison_stack.pop()
    assert popped is self._sem_poison
    # skip clear_and_free_semaphores: its EVENT_SEMAPHORE_RANGE_CLEAR InstISA
    # fails codegen here, and this kernel is built fresh per NEFF load so the
    # semaphores are never recycled.
    nc.all_engine_barrier()


def build_bass(cut=99):
    import types

    nc = bass.Bass()
    x_in = nc.declare_dram_parameter("x_loc", [BPC, N, D], F32, isOutput=False)
    w_in = nc.declare_dram_parameter("w_in", [D, K], F32, isOutput=False)
    c_in = nc.declare_dram_parameter("c_in", [D, K], F32, isOutput=False)
    y_out = nc.declare_dram_parameter("y_loc", [BPC, D * K], F32, isOutput=True)
    with ExitStack() as ctx:
        tc = ctx.enter_context(tile.TileContext(nc))
        tc._drain_and_barrier = types.MethodType(_split_drain_and_barrier, tc)
        _emit(ctx, tc, y_out, x_in, w_in, c_in)
    # strip same-engine self-waits from multi-wait instructions: the engines
    # dispatch in FIFO order and DVE/ACT drain between ops, so a self-wait
    # whose target precedes in the same stream guards only pseudo-hazards
    # (PSUM bank read-read serialization); walrus allows one wait only.
    eng_name = {mybir.EngineType.Activation: "Activation",
                mybir.EngineType.PE: "PE",
                mybir.EngineType.DVE: "DVE",
                mybir.EngineType.Pool: "Pool",
                mybir.EngineType.SP: "SP"}
    for name, inst in nc.inst_map.items():
        si = inst.sync_info
        if si is None or len(si.on_wait) <= 1:
            continue
        en = eng_name.get(getattr(inst, "engine", None))
        if en is None:
            continue
        keep = [w for w in si.on_wait if not w.ant_name.startswith(en + "_")]
        if 0 < len(keep) < len(si.on_wait):
            inst.sync_info = mybir.SyncInfo(on_wait=keep,
                                            on_update=list(si.on_update))
    return nc


def run(x, W, C, trace=False, tmpdir=None):
    from concourse.bass_utils import run_bass_kernel_spmd

    x = np.ascontiguousarray(x, dtype=np.float32).reshape(B, N, D)
    W = np.ascontiguousarray(W, dtype=np.float32)
    C = np.ascontiguousarray(C, dtype=np.float32)

    nc = build_bass()
    in_maps = [
        {"x_loc": x[c * BPC:(c + 1) * BPC], "w_in": W, "c_in": C}
        for c in range(NCORES)
    ]
    res = run_bass_kernel_spmd(nc, in_maps, list(range(NCORES)), trace=trace,
                               tmpdir=tmpdir)
    y = np.concatenate([res.results[c]["y_loc"] for c in range(NCORES)], axis=0)
    return y.astype(np.float32), res


def kernel(x, W, C):
    y, _ = run(x, W, C, trace=False)
    return y
